# revision 4
# baseline (speedup 1.0000x reference)
"""MoE top-2 routing kernel for 8 Trainium2 NeuronCores.

Strategy (expert-parallel with host-side dispatch):
  - Router (x @ w_router, softmax, top-2, combine weights) computed on host:
    it is 0.1% of the FLOPs and produces the dispatch indices needed to
    shard the tokens anyway.
  - Each of the 6 experts' FFN (D=1024 -> H=4096 -> D=1024) is split 4-ways
    along the hidden dim H into 24 shards of (1024 -> 1024 -> 1024).
    24 shards / 8 cores = 3 shards per core, perfectly weight-balanced.
  - Tokens routed to expert e (gathered, transposed to [D, C] feature-major,
    zero-padded to common capacity C) are processed by all 4 of e's shards;
    each shard produces a partial y^T[D, C] (sum over its H quarter).
  - Host sums the 4 partials per expert, scales by the top-2 combine weight
    and scatter-adds into the output.
  - Device kernel per core: 3x dense fused MLP: h^T = gelu(w1s^T x^T) tile
    by tile, y^T = w2s^T h^T, all fp32 (PE truncates to ~fp22 internally).
"""

import functools

import numpy as np

import concourse.bacc as bacc
import concourse.bass as bass
import concourse.mybir as mybir
import concourse.tile as tile
from concourse.bass_utils import run_bass_kernel_spmd

N_EXPERTS = 6
TOP_K = 2
AUX_COEFF = 0.01
B, T, D, H = 4, 2048, 1024, 4096
N_TOKENS = B * T
N_CORES = 8
N_SPLIT = 4                     # H split per expert
H_SH = H // N_SPLIT             # 1024
N_SHARDS = N_EXPERTS * N_SPLIT  # 24
S_PER_CORE = N_SHARDS // N_CORES  # 3
P = 128
FREE = 512                      # matmul moving free dim / PSUM bank width

# Populated by kernel() with the BassKernelResults of the last device run so
# a test harness can read exec_time_ns when BASS_TRACE=1 is set.
LAST_RESULTS = None


@functools.cache
def _build(C: int) -> bass.Bass:
    """Bass program for one core: 3 independent (1024 -> 1024 -> 1024) dense
    MLP shards over C tokens each, fp32, feature-major activations."""
    f32 = mybir.dt.float32
    KD = D // P      # 8 contraction chunks for layer 1
    KH = H_SH // P   # 8 contraction chunks for layer 2
    widths = [FREE] * (C // FREE)
    if C % FREE:
        widths.append(C % FREE)

    nc = bacc.Bacc()
    xT = nc.dram_tensor("xT", [S_PER_CORE, D, C], f32, kind="ExternalInput")
    w1s = nc.dram_tensor("w1s", [S_PER_CORE, D, H_SH], f32, kind="ExternalInput")
    w2s = nc.dram_tensor("w2s", [S_PER_CORE, H_SH, D], f32, kind="ExternalInput")
    yT = nc.dram_tensor("yT", [S_PER_CORE, D, C], f32, kind="ExternalOutput")

    with tile.TileContext(nc) as tc:
        with (
            tc.tile_pool(name="w1p", bufs=KD + 2) as w1p,
            tc.tile_pool(name="w2p", bufs=KH + 2) as w2p,
            tc.tile_pool(name="xp", bufs=KD + 4) as xp,
            tc.tile_pool(name="hp", bufs=KH + 4) as hp,
            tc.tile_pool(name="yp", bufs=4) as yp,
            tc.tile_pool(name="psh", bufs=3, space="PSUM") as psh,
            tc.tile_pool(name="psy", bufs=3, space="PSUM") as psy,
        ):
            for s in range(S_PER_CORE):
                w1t = []
                for k in range(KD):
                    w1k = w1p.tile([P, H_SH], f32, tag="w1", name=f"w1_{s}_{k}")
                    nc.sync.dma_start(w1k[:], w1s[s, k * P:(k + 1) * P, :])
                    w1t.append(w1k)
                w2t = []
                for k in range(KH):
                    w2k = w2p.tile([P, D], f32, tag="w2", name=f"w2_{s}_{k}")
                    nc.sync.dma_start(w2k[:], w2s[s, k * P:(k + 1) * P, :])
                    w2t.append(w2k)

                col = 0
                for n, nw in enumerate(widths):
                    nsl = slice(col, col + nw)
                    col += nw
                    xt = []
                    for k in range(KD):
                        xk = xp.tile([P, FREE], f32, tag="x", name=f"x_{s}_{n}_{k}")
                        nc.sync.dma_start(xk[:, :nw], xT[s, k * P:(k + 1) * P, nsl])
                        xt.append(xk)
                    ht = []
                    for m in range(KH):
                        ph = psh.tile([P, FREE], f32, tag="ps_h", name=f"ph_{s}_{n}_{m}")
                        for k in range(KD):
                            nc.tensor.matmul(
                                ph[:, :nw],
                                w1t[k][:, m * P:(m + 1) * P],
                                xt[k][:, :nw],
                                start=(k == 0),
                                stop=(k == KD - 1),
                            )
                        hm = hp.tile([P, FREE], f32, tag="h", name=f"h_{s}_{n}_{m}")
                        nc.scalar.activation(
                            hm[:, :nw], ph[:, :nw], mybir.ActivationFunctionType.Gelu
                        )
                        ht.append(hm)
                    for d in range(KD):
                        py = psy.tile([P, FREE], f32, tag="ps_y", name=f"py_{s}_{n}_{d}")
                        for k in range(KH):
                            nc.tensor.matmul(
                                py[:, :nw],
                                w2t[k][:, d * P:(d + 1) * P],
                                ht[k][:, :nw],
                                start=(k == 0),
                                stop=(k == KH - 1),
                            )
                        yd = yp.tile([P, FREE], f32, tag="y", name=f"y_{s}_{n}_{d}")
                        nc.vector.tensor_copy(yd[:, :nw], py[:, :nw])
                        nc.sync.dma_start(yT[s, d * P:(d + 1) * P, nsl], yd[:, :nw])
    nc.finalize()
    return nc


def _route(xf: np.ndarray, w_router: np.ndarray):
    """Host router: softmax probs (float64 for stable ordering), top-2
    indices and renormalized combine weights, aux loss."""
    logits = xf.astype(np.float64) @ w_router.astype(np.float64)
    z = logits - logits.max(axis=-1, keepdims=True)
    p = np.exp(z)
    p /= p.sum(axis=-1, keepdims=True)

    ar = np.arange(xf.shape[0])
    top1 = p.argmax(axis=-1)
    pm = p.copy()
    pm[ar, top1] = -np.inf
    top2 = pm.argmax(axis=-1)
    p1 = p[ar, top1]
    p2 = p[ar, top2]
    c1 = p1 / (p1 + p2)
    c2 = p2 / (p1 + p2)

    tokens_per_expert = p.mean(axis=0)
    aux = AUX_COEFF * np.mean((tokens_per_expert - 1.0 / N_EXPERTS) ** 2)
    return top1, top2, c1, c2, np.float32(aux)


def kernel(x, w_router, w1, w2):
    global LAST_RESULTS
    x = np.asarray(x, dtype=np.float32)
    w_router = np.asarray(w_router, dtype=np.float32)
    w1 = np.asarray(w1, dtype=np.float32)
    w2 = np.asarray(w2, dtype=np.float32)

    xf = x.reshape(N_TOKENS, D)
    top1, top2, c1, c2, aux = _route(xf, w_router)

    # Gather tokens per expert.
    idx = [np.where((top1 == e) | (top2 == e))[0] for e in range(N_EXPERTS)]
    comb = [
        np.where(top1[idx[e]] == e, c1[idx[e]], c2[idx[e]]).astype(np.float32)
        for e in range(N_EXPERTS)
    ]
    counts = [len(i) for i in idx]
    C = max(128, -(-max(counts) // P) * P)  # capacity, multiple of 128

    xT_e = []
    for e in range(N_EXPERTS):
        g = np.zeros((D, C), dtype=np.float32)
        g[:, :counts[e]] = xf[idx[e]].T
        xT_e.append(g)

    # Shard (e, q) -> core sid // S_PER_CORE, slot sid % S_PER_CORE.
    in_maps = []
    for core in range(N_CORES):
        xT = np.empty((S_PER_CORE, D, C), dtype=np.float32)
        w1s = np.empty((S_PER_CORE, D, H_SH), dtype=np.float32)
        w2s = np.empty((S_PER_CORE, H_SH, D), dtype=np.float32)
        for j in range(S_PER_CORE):
            sid = core * S_PER_CORE + j
            e, q = divmod(sid, N_SPLIT)
            xT[j] = xT_e[e]
            w1s[j] = w1[e][:, q * H_SH:(q + 1) * H_SH]
            w2s[j] = w2[e][q * H_SH:(q + 1) * H_SH, :]
        in_maps.append({"xT": xT, "w1s": w1s, "w2s": w2s})

    nc = _build(C)
    res = run_bass_kernel_spmd(nc, in_maps, core_ids=list(range(N_CORES)))
    LAST_RESULTS = res

    out = np.zeros((N_TOKENS, D), dtype=np.float32)
    for e in range(N_EXPERTS):
        acc = np.zeros((D, counts[e]), dtype=np.float32)
        for q in range(N_SPLIT):
            sid = e * N_SPLIT + q
            core, j = divmod(sid, S_PER_CORE)
            acc += res.results[core]["yT"][j][:, :counts[e]]
        out[idx[e]] += comb[e][:, None] * acc.T

    return out.reshape(B, T, D), aux


# revision 5
# speedup vs baseline: 3.3675x; 3.3675x over previous
"""MoE top-2 routing kernel for 8 Trainium2 NeuronCores.

Strategy (expert-parallel with host-side dispatch):
  - Router (x @ w_router, softmax, top-2, combine weights) computed on host:
    it is 0.1% of the FLOPs and produces the dispatch indices needed to
    shard the tokens anyway.
  - Each of the 6 experts' FFN (D=1024 -> H=4096 -> D=1024) is split 4-ways
    along the hidden dim H into 24 shards of (1024 -> 1024 -> 1024).
    24 shards / 8 cores = 3 shards per core, perfectly weight-balanced.
  - Tokens routed to expert e (gathered, transposed to [D, C] feature-major,
    zero-padded to common capacity C) are processed by all 4 of e's shards;
    each shard produces a partial y^T[D, C] (sum over its H quarter).
  - Host sums the 4 partials per expert, scales by the top-2 combine weight
    and scatter-adds into the output.
  - Device kernel per core: 3x dense fused MLP: h^T = gelu(w1s^T x^T) tile
    by tile, y^T = w2s^T h^T, all fp32 (PE truncates to ~fp22 internally).
"""

import functools

import numpy as np

import concourse.bacc as bacc
import concourse.bass as bass
import concourse.mybir as mybir
import concourse.tile as tile
from concourse.bass_utils import run_bass_kernel_spmd

N_EXPERTS = 6
TOP_K = 2
AUX_COEFF = 0.01
B, T, D, H = 4, 2048, 1024, 4096
N_TOKENS = B * T
N_CORES = 8
N_SPLIT = 4                     # H split per expert
H_SH = H // N_SPLIT             # 1024
N_SHARDS = N_EXPERTS * N_SPLIT  # 24
S_PER_CORE = N_SHARDS // N_CORES  # 3
P = 128
FREE = 512                      # matmul moving free dim / PSUM bank width

# Populated by kernel() with the BassKernelResults of the last device run so
# a test harness can read exec_time_ns when BASS_TRACE=1 is set.
LAST_RESULTS = None


@functools.cache
def _build(C: int) -> bass.Bass:
    """Bass program for one core: 3 independent (1024 -> 1024 -> 1024) dense
    MLP shards over C tokens each, fp32, feature-major activations."""
    f32 = mybir.dt.float32
    f32r = mybir.dt.float32r
    KD = D // P      # 8 contraction chunks for layer 1
    KH = H_SH // P   # 8 contraction chunks for layer 2
    widths = [FREE] * (C // FREE)
    if C % FREE:
        widths.append(C % FREE)

    nc = bacc.Bacc()
    xT = nc.dram_tensor("xT", [S_PER_CORE, D, C], f32r, kind="ExternalInput")
    w1s = nc.dram_tensor("w1s", [S_PER_CORE, D, H_SH], f32r, kind="ExternalInput")
    w2s = nc.dram_tensor("w2s", [S_PER_CORE, H_SH, D], f32r, kind="ExternalInput")
    yT = nc.dram_tensor("yT", [S_PER_CORE, D, C], f32r, kind="ExternalOutput")

    with tile.TileContext(nc) as tc:
        with (
            tc.tile_pool(name="w1p", bufs=KD + 2) as w1p,
            tc.tile_pool(name="w2p", bufs=KH + 2) as w2p,
            tc.tile_pool(name="xp", bufs=KD + 4) as xp,
            tc.tile_pool(name="hp", bufs=KH + 4) as hp,
            tc.tile_pool(name="yp", bufs=4) as yp,
            tc.tile_pool(name="psh", bufs=3, space="PSUM") as psh,
            tc.tile_pool(name="psy", bufs=3, space="PSUM") as psy,
        ):
            for s in range(S_PER_CORE):
                w1t = []
                for k in range(KD):
                    w1k = w1p.tile([P, H_SH], f32r, tag="w1", name=f"w1_{s}_{k}")
                    nc.sync.dma_start(w1k[:], w1s[s, k * P:(k + 1) * P, :])
                    w1t.append(w1k)
                w2t = []
                for k in range(KH):
                    w2k = w2p.tile([P, D], f32r, tag="w2", name=f"w2_{s}_{k}")
                    nc.sync.dma_start(w2k[:], w2s[s, k * P:(k + 1) * P, :])
                    w2t.append(w2k)

                col = 0
                for n, nw in enumerate(widths):
                    nsl = slice(col, col + nw)
                    col += nw
                    xt = []
                    for k in range(KD):
                        xk = xp.tile([P, FREE], f32r, tag="x", name=f"x_{s}_{n}_{k}")
                        nc.sync.dma_start(xk[:, :nw], xT[s, k * P:(k + 1) * P, nsl])
                        xt.append(xk)
                    ht = []
                    for m in range(KH):
                        ph = psh.tile([P, FREE], f32, tag="ps_h", name=f"ph_{s}_{n}_{m}")
                        for k in range(KD):
                            nc.tensor.matmul(
                                ph[:, :nw],
                                w1t[k][:, m * P:(m + 1) * P],
                                xt[k][:, :nw],
                                start=(k == 0),
                                stop=(k == KD - 1),
                            )
                        hm = hp.tile([P, FREE], f32r, tag="h", name=f"h_{s}_{n}_{m}")
                        nc.scalar.activation(
                            hm[:, :nw], ph[:, :nw], mybir.ActivationFunctionType.Gelu
                        )
                        ht.append(hm)
                    for d in range(KD):
                        py = psy.tile([P, FREE], f32, tag="ps_y", name=f"py_{s}_{n}_{d}")
                        for k in range(KH):
                            nc.tensor.matmul(
                                py[:, :nw],
                                w2t[k][:, d * P:(d + 1) * P],
                                ht[k][:, :nw],
                                start=(k == 0),
                                stop=(k == KH - 1),
                            )
                        yd = yp.tile([P, FREE], f32r, tag="y", name=f"y_{s}_{n}_{d}")
                        nc.vector.tensor_copy(yd[:, :nw], py[:, :nw])
                        nc.sync.dma_start(yT[s, d * P:(d + 1) * P, nsl], yd[:, :nw])
    nc.finalize()
    return nc


def _route(xf: np.ndarray, w_router: np.ndarray):
    """Host router: softmax probs (float64 for stable ordering), top-2
    indices and renormalized combine weights, aux loss."""
    logits = xf.astype(np.float64) @ w_router.astype(np.float64)
    z = logits - logits.max(axis=-1, keepdims=True)
    p = np.exp(z)
    p /= p.sum(axis=-1, keepdims=True)

    ar = np.arange(xf.shape[0])
    top1 = p.argmax(axis=-1)
    pm = p.copy()
    pm[ar, top1] = -np.inf
    top2 = pm.argmax(axis=-1)
    p1 = p[ar, top1]
    p2 = p[ar, top2]
    c1 = p1 / (p1 + p2)
    c2 = p2 / (p1 + p2)

    tokens_per_expert = p.mean(axis=0)
    aux = AUX_COEFF * np.mean((tokens_per_expert - 1.0 / N_EXPERTS) ** 2)
    return top1, top2, c1, c2, np.float32(aux)


def kernel(x, w_router, w1, w2):
    global LAST_RESULTS
    x = np.asarray(x, dtype=np.float32)
    w_router = np.asarray(w_router, dtype=np.float32)
    w1 = np.asarray(w1, dtype=np.float32)
    w2 = np.asarray(w2, dtype=np.float32)

    xf = x.reshape(N_TOKENS, D)
    top1, top2, c1, c2, aux = _route(xf, w_router)

    # Gather tokens per expert.
    idx = [np.where((top1 == e) | (top2 == e))[0] for e in range(N_EXPERTS)]
    comb = [
        np.where(top1[idx[e]] == e, c1[idx[e]], c2[idx[e]]).astype(np.float32)
        for e in range(N_EXPERTS)
    ]
    counts = [len(i) for i in idx]
    C = max(128, -(-max(counts) // P) * P)  # capacity, multiple of 128

    xT_e = []
    for e in range(N_EXPERTS):
        g = np.zeros((D, C), dtype=np.float32)
        g[:, :counts[e]] = xf[idx[e]].T
        xT_e.append(g)

    # Shard (e, q) -> core sid // S_PER_CORE, slot sid % S_PER_CORE.
    in_maps = []
    for core in range(N_CORES):
        xT = np.empty((S_PER_CORE, D, C), dtype=np.float32)
        w1s = np.empty((S_PER_CORE, D, H_SH), dtype=np.float32)
        w2s = np.empty((S_PER_CORE, H_SH, D), dtype=np.float32)
        for j in range(S_PER_CORE):
            sid = core * S_PER_CORE + j
            e, q = divmod(sid, N_SPLIT)
            xT[j] = xT_e[e]
            w1s[j] = w1[e][:, q * H_SH:(q + 1) * H_SH]
            w2s[j] = w2[e][q * H_SH:(q + 1) * H_SH, :]
        in_maps.append({"xT": xT, "w1s": w1s, "w2s": w2s})

    nc = _build(C)
    res = run_bass_kernel_spmd(nc, in_maps, core_ids=list(range(N_CORES)))
    LAST_RESULTS = res

    out = np.zeros((N_TOKENS, D), dtype=np.float32)
    for e in range(N_EXPERTS):
        acc = np.zeros((D, counts[e]), dtype=np.float32)
        for q in range(N_SPLIT):
            sid = e * N_SPLIT + q
            core, j = divmod(sid, S_PER_CORE)
            acc += res.results[core]["yT"][j][:, :counts[e]]
        out[idx[e]] += comb[e][:, None] * acc.T

    return out.reshape(B, T, D), aux


# revision 12
# speedup vs baseline: 3.8090x; 1.1311x over previous
"""MoE top-2 routing kernel for 8 Trainium2 NeuronCores.

Strategy (expert-parallel with host-side dispatch):
  - Router (x @ w_router, softmax, top-2, combine weights) computed on host:
    it is 0.1% of the FLOPs and produces the dispatch indices needed to
    shard the tokens anyway.
  - Each of the 6 experts' FFN (D=1024 -> H=4096 -> D=1024) is split 4-ways
    along the hidden dim H into 24 shards of (1024 -> 1024 -> 1024).
    24 shards / 8 cores = 3 shards per core, perfectly weight-balanced.
  - Tokens routed to expert e (gathered, transposed to [D, C] feature-major,
    zero-padded to common capacity C) are processed by all 4 of e's shards;
    each shard produces a partial y^T[D, C] (sum over its H quarter).
  - Host sums the 4 partials per expert, scales by the top-2 combine weight
    and scatter-adds into the output.
  - Device kernel per core: 3x dense fused MLP: h^T = gelu(w1s^T x^T) tile
    by tile, y^T = w2s^T h^T, all fp32 (PE truncates to ~fp22 internally).
"""

import functools
import time

import numpy as np

import concourse.bacc as bacc
import concourse.bass as bass
import concourse.mybir as mybir
import concourse.tile as tile
from concourse.bass_utils import run_bass_kernel_spmd

N_EXPERTS = 6
TOP_K = 2
AUX_COEFF = 0.01
B, T, D, H = 4, 2048, 1024, 4096
N_TOKENS = B * T
N_CORES = 8
N_SPLIT = 4                     # H split per expert
H_SH = H // N_SPLIT             # 1024
N_SHARDS = N_EXPERTS * N_SPLIT  # 24
S_PER_CORE = N_SHARDS // N_CORES  # 3
P = 128
FREE = 512                      # matmul moving free dim / PSUM bank width

# Populated by kernel() with the BassKernelResults of the last device run so
# a test harness can read exec_time_ns when BASS_TRACE=1 is set.
LAST_RESULTS = None


@functools.cache
def _build(C: int) -> bass.Bass:
    """Bass program for one core: 3 independent (1024 -> 1024 -> 1024) dense
    MLP shards over C tokens each, fp32, feature-major activations."""
    f32 = mybir.dt.float32
    f32r = mybir.dt.float32r
    KD = D // P      # 8 contraction chunks for layer 1
    KH = H_SH // P   # 8 contraction chunks for layer 2
    widths = [FREE] * (C // FREE)
    if C % FREE:
        widths.append(C % FREE)

    nc = bacc.Bacc()
    xT = nc.dram_tensor("xT", [S_PER_CORE, D, C], f32r, kind="ExternalInput")
    w1s = nc.dram_tensor("w1s", [S_PER_CORE, D, H_SH], f32r, kind="ExternalInput")
    w2s = nc.dram_tensor("w2s", [S_PER_CORE, H_SH, D], f32r, kind="ExternalInput")
    yT = nc.dram_tensor("yT", [S_PER_CORE, D, C], f32r, kind="ExternalOutput")

    with tile.TileContext(nc) as tc:
        with (
            tc.tile_pool(name="w1p", bufs=4 * KD) as w1p,
            tc.tile_pool(name="w2p", bufs=2 * KH) as w2p,
            tc.tile_pool(name="xp", bufs=KD + 2) as xp,
            tc.tile_pool(name="hp", bufs=KH + 2) as hp,
            tc.tile_pool(name="yp", bufs=4) as yp,
            tc.tile_pool(name="psh", bufs=3, space="PSUM") as psh,
            tc.tile_pool(name="psy", bufs=3, space="PSUM") as psy,
        ):
            def load_x(s, n, nsl, nw):
                xt = []
                for k in range(KD):
                    xk = xp.tile([P, FREE], f32r, tag="x", name=f"x_{s}_{n}_{k}")
                    nc.sync.dma_start(xk[:, :nw], xT[s, k * P:(k + 1) * P, nsl])
                    xt.append(xk)
                return xt

            for s in range(S_PER_CORE):
                # w1 loaded in half-width tiles, low halves first: the first
                # matmuls (m=0..3) need only cols 0:512 of each k chunk, so
                # compute starts after 2 MB instead of 4 MB of weight DMA.
                w1t = [[None, None] for _ in range(KD)]

                def load_w1_half(s, half):
                    csl = slice(half * (H_SH // 2), (half + 1) * (H_SH // 2))
                    for k in range(KD):
                        w1k = w1p.tile([P, H_SH // 2], f32r, tag="w1",
                                       name=f"w1_{s}_{k}_{half}")
                        nc.sync.dma_start(w1k[:], w1s[s, k * P:(k + 1) * P, csl])
                        w1t[k][half] = w1k

                # DMA issue order tracks first-use order: w1 low halves
                # (m=0..3), first n-tile's x, w1 high halves, then w2
                # (layer 2 starts ~15 us after layer 1).
                load_w1_half(s, 0)
                xt0 = load_x(s, 0, slice(0, widths[0]), widths[0])
                load_w1_half(s, 1)
                w2t = []
                for k in range(KH):
                    w2k = w2p.tile([P, D], f32r, tag="w2", name=f"w2_{s}_{k}")
                    nc.sync.dma_start(w2k[:], w2s[s, k * P:(k + 1) * P, :])
                    w2t.append(w2k)

                col = 0
                for n, nw in enumerate(widths):
                    nsl = slice(col, col + nw)
                    col += nw
                    xt = xt0 if n == 0 else load_x(s, n, nsl, nw)
                    ht = []
                    for m in range(KH):
                        ph = psh.tile([P, FREE], f32, tag="ps_h", name=f"ph_{s}_{n}_{m}")
                        for k in range(KD):
                            nc.tensor.matmul(
                                ph[:, :nw],
                                w1t[k][m // 4][:, (m % 4) * P:(m % 4 + 1) * P],
                                xt[k][:, :nw],
                                start=(k == 0),
                                stop=(k == KD - 1),
                            )
                        hm = hp.tile([P, FREE], f32r, tag="h", name=f"h_{s}_{n}_{m}")
                        nc.scalar.activation(
                            hm[:, :nw], ph[:, :nw], mybir.ActivationFunctionType.Gelu
                        )
                        ht.append(hm)
                    for d in range(KD):
                        py = psy.tile([P, FREE], f32, tag="ps_y", name=f"py_{s}_{n}_{d}")
                        for k in range(KH):
                            nc.tensor.matmul(
                                py[:, :nw],
                                w2t[k][:, d * P:(d + 1) * P],
                                ht[k][:, :nw],
                                start=(k == 0),
                                stop=(k == KH - 1),
                            )
                        yd = yp.tile([P, FREE], f32r, tag="y", name=f"y_{s}_{n}_{d}")
                        nc.vector.tensor_copy(yd[:, :nw], py[:, :nw])
                        nc.sync.dma_start(yT[s, d * P:(d + 1) * P, nsl], yd[:, :nw])
    nc.finalize()
    return nc


def _route(xf: np.ndarray, w_router: np.ndarray):
    """Host router: softmax probs (float64 for stable ordering), top-2
    indices and renormalized combine weights, aux loss."""
    logits = xf.astype(np.float64) @ w_router.astype(np.float64)
    z = logits - logits.max(axis=-1, keepdims=True)
    p = np.exp(z)
    p /= p.sum(axis=-1, keepdims=True)

    ar = np.arange(xf.shape[0])
    top1 = p.argmax(axis=-1)
    pm = p.copy()
    pm[ar, top1] = -np.inf
    top2 = pm.argmax(axis=-1)
    p1 = p[ar, top1]
    p2 = p[ar, top2]
    c1 = p1 / (p1 + p2)
    c2 = p2 / (p1 + p2)

    tokens_per_expert = p.mean(axis=0)
    aux = AUX_COEFF * np.mean((tokens_per_expert - 1.0 / N_EXPERTS) ** 2)
    return top1, top2, c1, c2, np.float32(aux)


def kernel(x, w_router, w1, w2):
    global LAST_RESULTS
    x = np.asarray(x, dtype=np.float32)
    w_router = np.asarray(w_router, dtype=np.float32)
    w1 = np.asarray(w1, dtype=np.float32)
    w2 = np.asarray(w2, dtype=np.float32)

    xf = x.reshape(N_TOKENS, D)
    top1, top2, c1, c2, aux = _route(xf, w_router)

    # Gather tokens per expert.
    idx = [np.where((top1 == e) | (top2 == e))[0] for e in range(N_EXPERTS)]
    comb = [
        np.where(top1[idx[e]] == e, c1[idx[e]], c2[idx[e]]).astype(np.float32)
        for e in range(N_EXPERTS)
    ]
    counts = [len(i) for i in idx]
    C = max(128, -(-max(counts) // P) * P)  # capacity, multiple of 128

    xT_e = []
    for e in range(N_EXPERTS):
        g = np.zeros((D, C), dtype=np.float32)
        g[:, :counts[e]] = xf[idx[e]].T
        xT_e.append(g)

    # Shard (e, q) -> core sid // S_PER_CORE, slot sid % S_PER_CORE.
    in_maps = []
    for core in range(N_CORES):
        xT = np.empty((S_PER_CORE, D, C), dtype=np.float32)
        w1s = np.empty((S_PER_CORE, D, H_SH), dtype=np.float32)
        w2s = np.empty((S_PER_CORE, H_SH, D), dtype=np.float32)
        for j in range(S_PER_CORE):
            sid = core * S_PER_CORE + j
            e, q = divmod(sid, N_SPLIT)
            xT[j] = xT_e[e]
            w1s[j] = w1[e][:, q * H_SH:(q + 1) * H_SH]
            w2s[j] = w2[e][q * H_SH:(q + 1) * H_SH, :]
        in_maps.append({"xT": xT, "w1s": w1s, "w2s": w2s})

    nc = _build(C)
    res = None
    for attempt in range(3):
        try:
            res = run_bass_kernel_spmd(nc, in_maps, core_ids=list(range(N_CORES)))
            break
        except Exception:
            if attempt == 2:
                raise
            time.sleep(5.0)
    LAST_RESULTS = res

    out = np.zeros((N_TOKENS, D), dtype=np.float32)
    for e in range(N_EXPERTS):
        acc = np.zeros((D, counts[e]), dtype=np.float32)
        for q in range(N_SPLIT):
            sid = e * N_SPLIT + q
            core, j = divmod(sid, S_PER_CORE)
            acc += res.results[core]["yT"][j][:, :counts[e]]
        out[idx[e]] += comb[e][:, None] * acc.T

    return out.reshape(B, T, D), aux


# revision 14
# speedup vs baseline: 3.8422x; 1.0087x over previous
"""MoE top-2 routing kernel for 8 Trainium2 NeuronCores.

Strategy (expert-parallel with host-side dispatch):
  - Router (x @ w_router, softmax, top-2, combine weights) computed on host:
    it is 0.1% of the FLOPs and produces the dispatch indices needed to
    shard the tokens anyway.
  - Each of the 6 experts' FFN (D=1024 -> H=4096 -> D=1024) is split 4-ways
    along the hidden dim H into 24 shards of (1024 -> 1024 -> 1024).
    24 shards / 8 cores = 3 shards per core, perfectly weight-balanced.
  - Tokens routed to expert e (gathered, transposed to [D, C] feature-major,
    zero-padded to common capacity C) are processed by all 4 of e's shards;
    each shard produces a partial y^T[D, C] (sum over its H quarter).
  - Host sums the 4 partials per expert, scales by the top-2 combine weight
    and scatter-adds into the output.
  - Device kernel per core: 3x dense fused MLP: h^T = gelu(w1s^T x^T) tile
    by tile, y^T = w2s^T h^T, all fp32 (PE truncates to ~fp22 internally).
"""

import functools
import time

import numpy as np

import concourse.bacc as bacc
import concourse.bass as bass
import concourse.mybir as mybir
import concourse.tile as tile
from concourse.bass_utils import run_bass_kernel_spmd

N_EXPERTS = 6
TOP_K = 2
AUX_COEFF = 0.01
B, T, D, H = 4, 2048, 1024, 4096
N_TOKENS = B * T
N_CORES = 8
N_SPLIT = 4                     # H split per expert
H_SH = H // N_SPLIT             # 1024
N_SHARDS = N_EXPERTS * N_SPLIT  # 24
S_PER_CORE = N_SHARDS // N_CORES  # 3
P = 128
FREE = 512                      # matmul moving free dim / PSUM bank width

# Populated by kernel() with the BassKernelResults of the last device run so
# a test harness can read exec_time_ns when BASS_TRACE=1 is set.
LAST_RESULTS = None


@functools.cache
def _build(C: int) -> bass.Bass:
    """Bass program for one core: 3 independent (1024 -> 1024 -> 1024) dense
    MLP shards over C tokens each, fp32, feature-major activations."""
    f32 = mybir.dt.float32
    f32r = mybir.dt.float32r
    KD = D // P      # 8 contraction chunks for layer 1
    KH = H_SH // P   # 8 contraction chunks for layer 2
    widths = [FREE] * (C // FREE)
    if C % FREE:
        widths.append(C % FREE)

    nc = bacc.Bacc()
    xT = nc.dram_tensor("xT", [S_PER_CORE, D, C], f32r, kind="ExternalInput")
    w1s = nc.dram_tensor("w1s", [S_PER_CORE, D, H_SH], f32r, kind="ExternalInput")
    w2s = nc.dram_tensor("w2s", [S_PER_CORE, H_SH, D], f32r, kind="ExternalInput")
    yT = nc.dram_tensor("yT", [S_PER_CORE, D, C], f32r, kind="ExternalOutput")

    # Pair the n-tiles: two 512-wide subtiles share one weight load per
    # (m, k), halving LDWEIGHTS pressure on the PE.
    pairs = []
    i = 0
    while i < len(widths):
        if i + 1 < len(widths):
            pairs.append((i, widths[i], i + 1, widths[i + 1]))
            i += 2
        else:
            pairs.append((i, widths[i], None, 0))
            i += 1

    with tile.TileContext(nc) as tc:
        with (
            tc.tile_pool(name="w1p", bufs=3 * KD) as w1p,
            tc.tile_pool(name="w2p", bufs=KH + KH // 2) as w2p,
            tc.tile_pool(name="xp", bufs=2 * KD + 2) as xp,
            tc.tile_pool(name="hp", bufs=2 * KH + 2) as hp,
            tc.tile_pool(name="yp", bufs=4) as yp,
            tc.tile_pool(name="psh", bufs=4, space="PSUM") as psh,
            tc.tile_pool(name="psy", bufs=4, space="PSUM") as psy,
        ):
            def load_x(s, n, nsl, nw):
                xt = []
                for k in range(KD):
                    xk = xp.tile([P, FREE], f32r, tag="x", name=f"x_{s}_{n}_{k}")
                    nc.sync.dma_start(xk[:, :nw], xT[s, k * P:(k + 1) * P, nsl])
                    xt.append(xk)
                return xt

            for s in range(S_PER_CORE):
                # w1 loaded in half-width tiles, low halves first: the first
                # matmuls (m=0..3) need only cols 0:512 of each k chunk, so
                # compute starts after 2 MB instead of 4 MB of weight DMA.
                w1t = [[None, None] for _ in range(KD)]

                def load_w1_half(s, half):
                    csl = slice(half * (H_SH // 2), (half + 1) * (H_SH // 2))
                    for k in range(KD):
                        w1k = w1p.tile([P, H_SH // 2], f32r, tag="w1",
                                       name=f"w1_{s}_{k}_{half}")
                        nc.sync.dma_start(w1k[:], w1s[s, k * P:(k + 1) * P, csl])
                        w1t[k][half] = w1k

                # DMA issue order tracks first-use order: w1 low halves
                # (m=0..3), first n-pair's x, w1 high halves, then w2
                # (layer 2 starts ~15 us after layer 1).
                load_w1_half(s, 0)
                na0, nwa0, nb0, nwb0 = pairs[0]
                xta0 = load_x(s, na0, slice(0, nwa0), nwa0)
                xtb0 = load_x(s, nb0, slice(nwa0, nwa0 + nwb0), nwb0) if nb0 is not None else None
                load_w1_half(s, 1)
                w2t = []
                for k in range(KH):
                    w2k = w2p.tile([P, D], f32r, tag="w2", name=f"w2_{s}_{k}")
                    nc.sync.dma_start(w2k[:], w2s[s, k * P:(k + 1) * P, :])
                    w2t.append(w2k)

                for pi, (na, nwa, nb, nwb) in enumerate(pairs):
                    ca = na * FREE
                    cb = ca + nwa
                    sla = slice(ca, ca + nwa)
                    slb = slice(cb, cb + nwb)
                    if pi == 0:
                        xta, xtb = xta0, xtb0
                    else:
                        xta = load_x(s, na, sla, nwa)
                        xtb = load_x(s, nb, slb, nwb) if nb is not None else None
                    hta, htb = [], []
                    for m in range(KH):
                        pha = psh.tile([P, FREE], f32, tag="ps_h", name=f"pha_{s}_{na}_{m}")
                        phb = (psh.tile([P, FREE], f32, tag="ps_h", name=f"phb_{s}_{na}_{m}")
                               if xtb is not None else None)
                        for k in range(KD):
                            w1sl = w1t[k][m // 4][:, (m % 4) * P:(m % 4 + 1) * P]
                            nc.tensor.matmul(pha[:, :nwa], w1sl, xta[k][:, :nwa],
                                             start=(k == 0), stop=(k == KD - 1))
                            if phb is not None:
                                nc.tensor.matmul(phb[:, :nwb], w1sl, xtb[k][:, :nwb],
                                                 start=(k == 0), stop=(k == KD - 1))
                        hma = hp.tile([P, FREE], f32r, tag="h", name=f"hma_{s}_{na}_{m}")
                        nc.scalar.activation(hma[:, :nwa], pha[:, :nwa],
                                             mybir.ActivationFunctionType.Gelu)
                        hta.append(hma)
                        if phb is not None:
                            hmb = hp.tile([P, FREE], f32r, tag="h", name=f"hmb_{s}_{na}_{m}")
                            nc.scalar.activation(hmb[:, :nwb], phb[:, :nwb],
                                                 mybir.ActivationFunctionType.Gelu)
                            htb.append(hmb)
                    for d in range(KD):
                        pya = psy.tile([P, FREE], f32, tag="ps_y", name=f"pya_{s}_{na}_{d}")
                        pyb = (psy.tile([P, FREE], f32, tag="ps_y", name=f"pyb_{s}_{na}_{d}")
                               if xtb is not None else None)
                        for k in range(KH):
                            w2sl = w2t[k][:, d * P:(d + 1) * P]
                            nc.tensor.matmul(pya[:, :nwa], w2sl, hta[k][:, :nwa],
                                             start=(k == 0), stop=(k == KH - 1))
                            if pyb is not None:
                                nc.tensor.matmul(pyb[:, :nwb], w2sl, htb[k][:, :nwb],
                                                 start=(k == 0), stop=(k == KH - 1))
                        yda = yp.tile([P, FREE], f32r, tag="y", name=f"yda_{s}_{na}_{d}")
                        nc.vector.tensor_copy(yda[:, :nwa], pya[:, :nwa])
                        nc.sync.dma_start(yT[s, d * P:(d + 1) * P, sla], yda[:, :nwa])
                        if pyb is not None:
                            ydb = yp.tile([P, FREE], f32r, tag="y", name=f"ydb_{s}_{na}_{d}")
                            nc.vector.tensor_copy(ydb[:, :nwb], pyb[:, :nwb])
                            nc.sync.dma_start(yT[s, d * P:(d + 1) * P, slb], ydb[:, :nwb])
    nc.finalize()
    return nc


def _route(xf: np.ndarray, w_router: np.ndarray):
    """Host router: softmax probs (float64 for stable ordering), top-2
    indices and renormalized combine weights, aux loss."""
    logits = xf.astype(np.float64) @ w_router.astype(np.float64)
    z = logits - logits.max(axis=-1, keepdims=True)
    p = np.exp(z)
    p /= p.sum(axis=-1, keepdims=True)

    ar = np.arange(xf.shape[0])
    top1 = p.argmax(axis=-1)
    pm = p.copy()
    pm[ar, top1] = -np.inf
    top2 = pm.argmax(axis=-1)
    p1 = p[ar, top1]
    p2 = p[ar, top2]
    c1 = p1 / (p1 + p2)
    c2 = p2 / (p1 + p2)

    tokens_per_expert = p.mean(axis=0)
    aux = AUX_COEFF * np.mean((tokens_per_expert - 1.0 / N_EXPERTS) ** 2)
    return top1, top2, c1, c2, np.float32(aux)


def kernel(x, w_router, w1, w2):
    global LAST_RESULTS
    x = np.asarray(x, dtype=np.float32)
    w_router = np.asarray(w_router, dtype=np.float32)
    w1 = np.asarray(w1, dtype=np.float32)
    w2 = np.asarray(w2, dtype=np.float32)

    xf = x.reshape(N_TOKENS, D)
    top1, top2, c1, c2, aux = _route(xf, w_router)

    # Gather tokens per expert.
    idx = [np.where((top1 == e) | (top2 == e))[0] for e in range(N_EXPERTS)]
    comb = [
        np.where(top1[idx[e]] == e, c1[idx[e]], c2[idx[e]]).astype(np.float32)
        for e in range(N_EXPERTS)
    ]
    counts = [len(i) for i in idx]
    C = max(128, -(-max(counts) // P) * P)  # capacity, multiple of 128

    xT_e = []
    for e in range(N_EXPERTS):
        g = np.zeros((D, C), dtype=np.float32)
        g[:, :counts[e]] = xf[idx[e]].T
        xT_e.append(g)

    # Shard (e, q) -> core sid // S_PER_CORE, slot sid % S_PER_CORE.
    in_maps = []
    for core in range(N_CORES):
        xT = np.empty((S_PER_CORE, D, C), dtype=np.float32)
        w1s = np.empty((S_PER_CORE, D, H_SH), dtype=np.float32)
        w2s = np.empty((S_PER_CORE, H_SH, D), dtype=np.float32)
        for j in range(S_PER_CORE):
            sid = core * S_PER_CORE + j
            e, q = divmod(sid, N_SPLIT)
            xT[j] = xT_e[e]
            w1s[j] = w1[e][:, q * H_SH:(q + 1) * H_SH]
            w2s[j] = w2[e][q * H_SH:(q + 1) * H_SH, :]
        in_maps.append({"xT": xT, "w1s": w1s, "w2s": w2s})

    nc = _build(C)
    res = None
    for attempt in range(3):
        try:
            res = run_bass_kernel_spmd(nc, in_maps, core_ids=list(range(N_CORES)))
            break
        except Exception:
            if attempt == 2:
                raise
            time.sleep(5.0)
    LAST_RESULTS = res

    out = np.zeros((N_TOKENS, D), dtype=np.float32)
    for e in range(N_EXPERTS):
        acc = np.zeros((D, counts[e]), dtype=np.float32)
        for q in range(N_SPLIT):
            sid = e * N_SPLIT + q
            core, j = divmod(sid, S_PER_CORE)
            acc += res.results[core]["yT"][j][:, :counts[e]]
        out[idx[e]] += comb[e][:, None] * acc.T

    return out.reshape(B, T, D), aux
